# revision 53
# baseline (speedup 1.0000x reference)
"""Trainium2 Bass kernel for nn_Backflow (gnn_message_passing).

Math: res_i = xi(|x_i|, t) * x_i + sum_j eta(|x_i - x_j|, t) * (x_i - x_j)

Key transformations:
  1. sum_j eta_ij * (x_i - x_j) = S_i * x_i - T_i with S_i = sum_j eta_ij,
     T_i = sum_j eta_ij x_j — the (n,n,3) rij tensor is never materialized
     and the diagonal term cancels exactly.
  2. t is a scalar, so eta(d, t) and xi(r, t) are univariate smooth
     functions; fit Chebyshev polynomials on the exact input domain
     (fit error ~1e-11 here, far below fp32 noise) and evaluate on-device
     with a few wide DVE/ACT ops (Estrin form).
  3. dist^2 via the Gram trick on the tensor engine:
     d2[j,i] = r2_j + r2_i - 2 x_j.x_i as one K=8 matmul per 128-j chunk.
     A tiny positive bias keeps rounding from driving diag d2 negative, so
     no relu clamp is needed (guarded by a host-side error-bound check).
  4. Grid is j-on-partitions so Eta slices are the matmul rhs for
     [T | S] = [x|1]^T @ Eta with a 4-column stationary operand (fast
     weight loads) and no transposes anywhere.

Sharding: row-block of 128 particles i per core (8 cores), x replicated.
"""

import numpy as np

N = 1024
NCORES = 8
PB = N // NCORES  # 128
# augji packed layout, 32 dense rows -> 4 SBUF quadrants (partitions 32g+0..7):
#   cols 0:128    lhsT chunk g          (aug_j columns for j-chunk g)
#   cols 128:256  lhsT chunk g+4
#   cols 256:384  rhs copy              (aug_i, replicated per quadrant)
# quadrant 0 extras: cols 384:512 rows 0-3 = xt4; 512:640 row 0 = wxi;
#   col 640 rows 0-3 = K correction
AUGW = 648  # (legacy name; MM block now bf16 in mm_d, extras in ext_d)

TRACE = False  # set by test harness to collect an NTFF profile
TRACE_DIR = None  # optional fixed dir for trace artifacts
LAST_PROFILE = None  # BassKernelResults of the last run (for test harness)

_PROG_CACHE = {}


def _fit_cheb(f, lo, hi, tol, max_deg=15):
    """Fit f on [lo, hi]; return power-basis coeffs in w = 2(d-lo)/(hi-lo)-1.

    Returned length is even (odd degree, zero-padded if needed).
    """
    from numpy.polynomial import chebyshev as C

    dd = np.linspace(lo, hi, 4001)
    ff = f(dd)
    ch = None
    for deg in [2] + list(range(3, max_deg + 1, 2)):
        ch = C.Chebyshev.fit(dd, ff, deg, domain=[lo, hi])
        if np.abs(ch(dd) - ff).max() < tol:
            break
    cw = C.cheb2poly(ch.coef)
    if len(cw) % 2:
        cw = np.append(cw, 0.0)
    return cw


class _PolyEmitter:
    """Estrin evaluation of sum_k cw[k] w^k over column slices of a grid.

    Input tile holds v = w + 1 (if in_is_v) or w directly. Powers of w^2
    go on ACT (Square) when use_act is set, else DVE tensor_mul.
    """

    def __init__(self, nc, mybir, pool, shape, cw, pfx, in_is_v, use_act,
                 neg1=None, zero=None):
        self.nc, self.mybir, self.pool = nc, mybir, pool
        self.shape, self.cw, self.pfx = shape, cw, pfx
        self.in_is_v, self.use_act = in_is_v, use_act
        self.neg1, self.zero = neg1, zero
        self.K = len(cw) // 2
        f32 = mybir.dt.float32
        self.tiles = {}
        nlv = 1
        k = self.K
        while k > 1:
            k = (k + 1) // 2
            nlv += 1

        def t(name):
            self.tiles[name] = pool.tile(
                shape, f32, tag=f"{pfx}{name}", name=f"{pfx}{name}"
            )

        for i in range(self.K):
            t(f"L{i}")
        lv, cnt = 1, self.K
        while cnt > 1:
            t(f"p{lv}")
            for i in range(0, cnt - 1, 2):
                t(f"q{lv}_{i}")
            cnt = (cnt + 1) // 2
            lv += 1

    def emit(self, v_tile, sl, final_out=None, eng=None, act_t0=False):
        """Emit ops for column slice sl; returns the tile holding the result.

        final_out: optional tile the last combine writes to (e.g. a bf16
        tile). eng: engine for the fast-cubic path (nc.vector or nc.gpsimd).
        Records the last ACT instruction in self.last_act_inst (None when
        the fast path is used — it needs no ACT at all).
        """
        nc, mybir, cw = self.nc, self.mybir, self.cw
        Alu = mybir.AluOpType
        Act = mybir.ActivationFunctionType
        T = self.tiles
        self.last_act_inst = None
        if eng is None:
            eng = nc.vector
        if (self.K == 2 and self.in_is_v and float(cw[0]) == 0.0
                and float(cw[3]) == 0.0):
            # centered quadratic: d_eta = (v-1) * (c1 + c2*(v-1)); no ACT
            c1, c2 = float(cw[1]), float(cw[2])
            A = T["L0"]
            nc.vector.tensor_scalar(A[:, sl], v_tile[:, sl], c2, c1 - c2,
                                    Alu.mult, Alu.add)
            dst = final_out if final_out is not None else T["q1_0"]
            nc.vector.scalar_tensor_tensor(dst[:, sl], v_tile[:, sl], 1.0,
                                           A[:, sl], Alu.subtract, Alu.mult)
            return dst
        if (self.K == 2 and self.in_is_v and self.use_act
                and float(cw[0]) == 0.0):
            # centered cubic: d_eta = (v-1) * (c1 + c2*(v-1) + c3*(v-1)^2)
            # w2 = (v-1)^2 on ACT; 3 ops on DVE
            c1, c2, c3 = float(cw[1]), float(cw[2]), float(cw[3])
            p = T["p1"]
            self.last_act_inst = nc.scalar.activation(
                p[:, sl], v_tile[:, sl], Act.Square,
                bias=self.neg1[: self.shape[0]],
            )
            t0 = T["L0"]
            self.t0_act_inst = None
            if act_t0:
                # ACT is idle after this half's Square; DVE is still busy
                self.t0_act_inst = nc.scalar.activation(
                    t0[:, sl], p[:, sl], Act.Copy,
                    bias=float(c1 - c2), scale=c3,
                )
            else:
                nc.vector.tensor_scalar(t0[:, sl], p[:, sl], c3, c1 - c2,
                                        Alu.mult, Alu.add)
            p2t = T["L1"]
            nc.vector.scalar_tensor_tensor(p2t[:, sl], v_tile[:, sl], c2,
                                           t0[:, sl], Alu.mult, Alu.add)
            dst = final_out if final_out is not None else T["q1_0"]
            nc.vector.scalar_tensor_tensor(dst[:, sl], v_tile[:, sl], 1.0,
                                           p2t[:, sl], Alu.subtract, Alu.mult)
            return dst
        cur = []
        for k in range(self.K):
            L = T[f"L{k}"]
            c1 = float(cw[2 * k + 1])
            c0 = float(cw[2 * k] - cw[2 * k + 1]) if self.in_is_v else float(cw[2 * k])
            nc.vector.tensor_scalar(L[:, sl], v_tile[:, sl], c1, c0, Alu.mult, Alu.add)
            cur.append(L)
        if self.K == 1:
            return cur[0]
        p = T["p1"]
        if self.use_act:
            bias = self.neg1 if self.in_is_v else self.zero
            self.last_act_inst = nc.scalar.activation(
                p[:, sl], v_tile[:, sl], Act.Square, bias=bias[: self.shape[0]]
            )
        else:
            if self.in_is_v:
                w = self.pool.tile(self.shape, self.mybir.dt.float32, tag=f"{self.pfx}w")
                nc.vector.tensor_scalar(w[:, sl], v_tile[:, sl], 1.0, -1.0, Alu.mult, Alu.add)
                nc.vector.tensor_mul(p[:, sl], w[:, sl], w[:, sl])
            else:
                nc.vector.tensor_mul(p[:, sl], v_tile[:, sl], v_tile[:, sl])
        lv = 1
        while len(cur) > 1:
            nxt = []
            last_level = len(cur) <= 2
            for i in range(0, len(cur) - 1, 2):
                q = T[f"q{lv}_{i}"]
                nc.vector.tensor_mul(q[:, sl], p[:, sl], cur[i + 1][:, sl])
                dst = final_out if (last_level and final_out is not None) else q
                nc.vector.tensor_add(dst[:, sl], cur[i][:, sl], q[:, sl])
                nxt.append(dst)
            if len(cur) % 2:
                nxt.append(cur[-1])
            cur = nxt
            if len(cur) > 1:
                p2 = T[f"p{lv + 1}"]
                if self.use_act:
                    self.last_act_inst = nc.scalar.activation(
                        p2[:, sl], p[:, sl], Act.Square, bias=self.zero[: self.shape[0]]
                    )
                else:
                    nc.vector.tensor_mul(p2[:, sl], p[:, sl], p[:, sl])
                p = p2
                lv += 1
        return cur[0]


def _build(cw_eta, cw_xi, s2):
    import concourse.bacc as bacc
    import concourse.bass as bass
    import concourse.mybir as mybir
    from concourse import tile

    f32 = mybir.dt.float32
    Alu = mybir.AluOpType
    Act = mybir.ActivationFunctionType

    nc = bacc.Bacc("TRN2", target_bir_lowering=False, debug=False)
    bf16 = mybir.dt.bfloat16
    # MM data: 4 quadrants x 16 rows (13 used: hi/lo split Gram operands)
    mm_d = nc.declare_dram_parameter("mm", [64, 384], bf16, isOutput=False)
    # extras: cols 0-2 x_i, col 3 wxi, row 0 cols 4:8 K row [0,K1,K2,K3]
    ext_d = nc.declare_dram_parameter("ext", [PB, 8], f32, isOutput=False)
    xaug_d = nc.declare_dram_parameter("xaug_r", [PB, 4 * NCORES], bf16, isOutput=False)
    out_d = nc.declare_dram_parameter("res", [PB, 3], f32, isOutput=True)

    with tile.TileContext(nc) as tc:
        with (
            tc.tile_pool(name="sb", bufs=1) as sb,
            tc.tile_pool(name="ps", bufs=1, space=bass.MemorySpace.PSUM) as ps,
        ):
            mmt = sb.tile([128, 384], bf16, tag="mmt")
            dma_engines = [nc.sync, nc.scalar, nc.gpsimd, nc.sync]
            for g in range(4):
                dma_engines[g].dma_start(
                    mmt[32 * g:32 * g + 16, :],
                    mm_d[16 * g:16 * g + 16, :],
                )
            # extras (x_i/wxi/K) needed late
            ext = sb.tile([PB, 8], f32, tag="ext")
            nc.sync.dma_start(ext[:], ext_d[:])
            xaug = sb.tile([PB, 4 * NCORES], bf16, tag="xaug")
            nc.scalar.dma_start(xaug[:], xaug_d[:])


            neg1 = sb.tile([128, 1], f32, tag="neg1")
            nc.vector.memset(neg1[:], -1.0)
            zero = sb.tile([128, 1], f32, tag="zero")
            nc.vector.memset(zero[:], 0.0)
            ones1 = sb.tile([1, PB], f32, tag="ones1")
            nc.vector.memset(ones1[:], 1.0)

            # dist^2 grid via 4x row tiling: quadrant g computes chunks g
            # and g+4 into its own PSUM bank; v cols = g*256 + (b//4)*128 + il
            d2t0 = ps.tile([128, 256], f32, tag="d2t0")
            d2t1 = ps.tile([128, 256], f32, tag="d2t1")
            d2t2 = ps.tile([128, 256], f32, tag="d2t2")
            d2t3 = ps.tile([128, 256], f32, tag="d2t3")
            d2 = [d2t0, d2t1, d2t2, d2t3]
            for g in range(4):
                for half in range(2):
                    # quadrant-grouped order [0,4],[1,5],.. so bank g is
                    # complete as early as possible for the sqrt chain.
                    # bf16 hi/lo-split operands: K=13, single HW pass.
                    nc.tensor.matmul(
                        d2[g][:, half * PB:(half + 1) * PB],
                        mmt[32 * g:32 * g + 13, half * 128:half * 128 + 128],
                        mmt[32 * g:32 * g + 13, 256:384],
                        start=True, stop=True,
                        tile_position=(32 * g, 0),
                    )

            v = sb.tile([128, N], f32, tag="v")
            etab = sb.tile([128, N], bf16, tag="etab")  # delta-eta in bf16
            # ts4 rows: 0 = S_delta (ones col), 1-3 = T_delta (x cols)
            ts4 = ps.tile([PB, 4], f32, tag="ts4")
            pe = _PolyEmitter(nc, mybir, sb, [128, N], cw_eta, "e",
                              in_is_v=True, use_act=True, neg1=neg1, zero=zero)
            prev_act = [None]

            def act_chain(inst):
                # pin ACT queue order (FIFO engine; Tile otherwise reorders)
                if prev_act[0] is not None:
                    tile.add_dep_helper(inst.ins, prev_act[0].ins, sync=False)
                prev_act[0] = inst

            first_ts = True
            for h in range(2):
                for g in (2 * h, 2 * h + 1):
                    # sqrt per bank (one [128,256] op per PSUM bank)
                    si = nc.scalar.activation(
                        v[:, g * 256:(g + 1) * 256], d2[g][:], Act.Sqrt,
                        bias=zero[:], scale=float(s2))
                    act_chain(si)
                # poly per half: wider DVE ops amortize per-op overhead
                pe.emit(v, slice(h * 512, (h + 1) * 512), final_out=etab,
                        act_t0=(h == 1))
                if pe.last_act_inst is not None:
                    act_chain(pe.last_act_inst)
                if getattr(pe, "t0_act_inst", None) is not None:
                    act_chain(pe.t0_act_inst)
                for g in (2 * h, 2 * h + 1):
                    for half in range(2):
                        b = g + 4 * half  # j-chunk index
                        col = g * 256 + half * 128
                        last = (h == 1 and g == 3 and half == 1)
                        nc.tensor.matmul(
                            ts4[:],
                            etab[:, col:col + PB],
                            xaug[:, 4 * b:4 * (b + 1)],
                            start=first_ts, stop=False,
                            skip_group_check=True,
                        )
                        first_ts = False

            # fold K into ts4 via a rank-1 matmul (lhsT=ones row, rhs=K row)
            nc.tensor.matmul(ts4[:], ones1[:], ext[0:1, 4:8],
                             start=False, stop=True, skip_group_check=True)
            # xi = c2*(w+h)^2 + k as ONE ACT Square (i-partitioned [128,1])
            assert len(cw_xi) == 4 and float(cw_xi[3]) == 0.0 and float(cw_xi[2]) > 0.0
            c0, c1, c2 = (float(cw_xi[0]), float(cw_xi[1]), float(cw_xi[2]))
            sq = c2 ** 0.5
            hh = c1 / (2.0 * c2)
            kk = c0 - c1 * c1 / (4.0 * c2)
            sxh = sb.tile([PB, 1], f32, tag="sxh")
            nc.vector.memset(sxh[:], sq * hh)
            xsq = sb.tile([PB, 1], f32, tag="xsq")
            si = nc.scalar.activation(xsq[:], ext[:, 3:4],
                                      Act.Square, bias=sxh[:], scale=sq)
            act_chain(si)
            # g = (xi_sq + k) + S   (S is ts4 col 0), per-partition scalar
            g = sb.tile([PB, 1], f32, tag="g")
            nc.vector.scalar_tensor_tensor(g[:], xsq[:], kk,
                                           ts4[:, 0:1], Alu.add, Alu.add)
            # res = x_i * g - (T_delta + K), one fused op, i-partitioned
            res = sb.tile([PB, 3], f32, tag="res")
            nc.vector.scalar_tensor_tensor(res[:], ext[:, 0:3], g[:],
                                           ts4[:, 1:4], Alu.mult, Alu.subtract)
            nc.sync.dma_start(out_d[:], res[:])

    nc.finalize()
    return nc


def _host_prep(x, r2_32, eps):
    aug_j = np.zeros((8, N), np.float32)
    aug_j[0:3] = -2.0 * x.T
    aug_j[3] = r2_32 + np.float32(eps)
    aug_j[4] = 1.0
    xaug_r = np.zeros((PB, 4 * NCORES), np.float32)
    for b in range(NCORES):
        xaug_r[:, 4 * b] = 1.0
        xaug_r[:, 4 * b + 1:4 * b + 4] = x[b * PB:(b + 1) * PB]
    return aug_j, xaug_r


def kernel(**inputs):
    global LAST_PROFILE
    x = np.ascontiguousarray(np.asarray(inputs["x"], dtype=np.float32))
    t = float(np.asarray(inputs["t"]))
    W = {
        k: np.asarray(v, np.float64)
        for k, v in inputs.items()
        if k not in ("x", "t")
    }

    def mlp(inp, p):
        sp = lambda z: np.logaddexp(0.0, z)
        h = sp(inp @ W[p + "_W1"] + W[p + "_b1"])
        h = sp(h @ W[p + "_W2"] + W[p + "_b2"])
        return h @ W[p + "_W3"] + W[p + "_b3"]

    def eta_f(dd):
        return mlp(np.stack([dd, np.full_like(dd, t)], -1), "eta")[..., 0]

    def xi_f(rr):
        return mlp(np.stack([rr, np.full_like(rr, t)], -1), "xi")[..., 0]

    r2_32 = (x * x).sum(1, dtype=np.float32)
    r64 = np.sqrt(r2_32.astype(np.float64))
    # eps shift keeps the PE-rounded diagonal of dist^2 positive (no relu).
    r2max = float(r2_32.max())
    # covers PE rounding + bf16 hi/lo split residuals (xl*xl term dropped)
    eps = max(2e-4 * max(r2max, 1.0), 1e-30)
    dmax = np.sqrt((2.0 * float(r64.max())) ** 2 + 2 * eps) * 1.0001 + 1e-12
    rlo = float(r64.min()) * 0.999 - 1e-12
    rhi = float(r64.max()) * 1.001 + 1e-12

    eta_scale = np.abs(eta_f(np.linspace(0, dmax, 257))).max()
    tol_eta = max(eta_scale * 1e-7, 1e-10)
    tol_xi = max(np.abs(xi_f(np.linspace(rlo, rhi, 257))).max() * 1e-7, 1e-10)
    # Guard for the eps shift: worst-case |eta'| * max d-shift must be tiny.
    dgrid = np.linspace(1e-3, dmax, 2049)
    deta = np.abs(np.gradient(eta_f(dgrid), dgrid)).max()
    dmin_guard = 1e-2  # conservative lower bound on off-diag distances
    shift_err = deta * eps / (2.0 * dmin_guard)
    assert shift_err < 1e-3 * max(eta_scale, 1e-30), (
        f"eps-shift error bound {shift_err} too large; need relu fallback"
    )

    cw_eta = _fit_cheb(eta_f, 0.0, dmax, tol_eta)
    cw_xi = _fit_cheb(xi_f, rlo, rhi, tol_xi)
    s = 2.0 / dmax
    s2 = s * s

    # Center-subtract eta so the on-device grid holds delta-eta (tiny values,
    # safe in bf16). The constant part is restored exactly: N*eta0 into the
    # xi constant (feeds S), eta0*sum_j x_j via the K column (feeds T).
    eta0 = float(cw_eta[0])
    cw_eta_dev = cw_eta.copy()
    cw_eta_dev[0] = 0.0
    cw_xi_dev = cw_xi.copy()
    cw_xi_dev[0] += N * eta0

    key = (cw_eta_dev.tobytes(), cw_xi_dev.tobytes(), float(s2))
    nc = _PROG_CACHE.get(key)
    if nc is None:
        nc = _build(cw_eta_dev, cw_xi_dev, s2)
        _PROG_CACHE[key] = nc

    aug_j, xaug_r = _host_prep(x, r2_32, eps)
    import ml_dtypes
    bf = ml_dtypes.bfloat16
    xaug_r = xaug_r.astype(bf)
    w_xi_full = (2.0 * (r64 - rlo) / (rhi - rlo) - 1.0).astype(np.float32)
    ksum = (eta0 * x.astype(np.float64).sum(0)).astype(np.float32)  # [3]
    # bf16 hi/lo splits for the single-pass Gram matmul
    xh = x.astype(bf)
    xl = (x - xh.astype(np.float32)).astype(bf)
    xh2 = (-2.0 * xh.astype(np.float32)).astype(bf)  # exact (exponent shift)
    xl2 = (-2.0 * xl.astype(np.float32)).astype(bf)
    r2e = (r2_32 + np.float32(eps)).astype(np.float32)
    r2eh = r2e.astype(bf)
    r2el = (r2e - r2eh.astype(np.float32)).astype(bf)
    r2h = r2_32.astype(bf)
    r2l = (r2_32 - r2h.astype(np.float32)).astype(bf)

    in_maps = []
    for m in range(NCORES):
        sl = slice(m * PB, (m + 1) * PB)
        mm = np.zeros((64, 384), bf)
        for g in range(4):
            R = 16 * g
            for half, c in ((0, g), (1, g + 4)):
                cs = slice(c * PB, (c + 1) * PB)
                col = slice(half * 128, (half + 1) * 128)
                mm[R + 0:R + 3, col] = xh2[cs].T
                mm[R + 3:R + 6, col] = xh2[cs].T
                mm[R + 6:R + 9, col] = xl2[cs].T
                mm[R + 9, col] = r2eh[cs]
                mm[R + 10, col] = r2el[cs]
                mm[R + 11, col] = 1.0
                mm[R + 12, col] = 1.0
            mm[R + 0:R + 3, 256:384] = xh[sl].T
            mm[R + 3:R + 6, 256:384] = xl[sl].T
            mm[R + 6:R + 9, 256:384] = xh[sl].T
            mm[R + 9, 256:384] = 1.0
            mm[R + 10, 256:384] = 1.0
            mm[R + 11, 256:384] = r2h[sl]
            mm[R + 12, 256:384] = r2l[sl]
        ext = np.zeros((PB, 8), np.float32)
        ext[:, 0:3] = x[sl]
        ext[:, 3] = w_xi_full[sl]
        ext[0, 5:8] = ksum
        in_maps.append({"mm": mm, "ext": ext, "xaug_r": xaug_r})

    from concourse.bass_utils import run_bass_kernel_spmd

    kw = {}
    if TRACE:
        kw = dict(trace=True, tmpdir=TRACE_DIR)
    out = run_bass_kernel_spmd(nc, in_maps, list(range(NCORES)), **kw)
    LAST_PROFILE = out
    res = np.concatenate(
        [out.results[m]["res"] for m in range(NCORES)], axis=0
    )
    return np.ascontiguousarray(res).astype(np.float32)


# revision 54
# speedup vs baseline: 1.0206x; 1.0206x over previous
"""Trainium2 Bass kernel for nn_Backflow (gnn_message_passing).

Math: res_i = xi(|x_i|, t) * x_i + sum_j eta(|x_i - x_j|, t) * (x_i - x_j)

Key transformations:
  1. sum_j eta_ij * (x_i - x_j) = S_i * x_i - T_i with S_i = sum_j eta_ij,
     T_i = sum_j eta_ij x_j — the (n,n,3) rij tensor is never materialized
     and the diagonal term cancels exactly.
  2. t is a scalar, so eta(d, t) and xi(r, t) are univariate smooth
     functions; fit Chebyshev polynomials on the exact input domain
     (fit error ~1e-11 here, far below fp32 noise) and evaluate on-device
     with a few wide DVE/ACT ops (Estrin form).
  3. dist^2 via the Gram trick on the tensor engine:
     d2[j,i] = r2_j + r2_i - 2 x_j.x_i as one K=8 matmul per 128-j chunk.
     A tiny positive bias keeps rounding from driving diag d2 negative, so
     no relu clamp is needed (guarded by a host-side error-bound check).
  4. Grid is j-on-partitions so Eta slices are the matmul rhs for
     [T | S] = [x|1]^T @ Eta with a 4-column stationary operand (fast
     weight loads) and no transposes anywhere.

Sharding: row-block of 128 particles i per core (8 cores), x replicated.
"""

import numpy as np

N = 1024
NCORES = 8
PB = N // NCORES  # 128
# augji packed layout, 32 dense rows -> 4 SBUF quadrants (partitions 32g+0..7):
#   cols 0:128    lhsT chunk g          (aug_j columns for j-chunk g)
#   cols 128:256  lhsT chunk g+4
#   cols 256:384  rhs copy              (aug_i, replicated per quadrant)
# quadrant 0 extras: cols 384:512 rows 0-3 = xt4; 512:640 row 0 = wxi;
#   col 640 rows 0-3 = K correction
AUGW = 648  # (legacy name; MM block now bf16 in mm_d, extras in ext_d)

TRACE = False  # set by test harness to collect an NTFF profile
TRACE_DIR = None  # optional fixed dir for trace artifacts
LAST_PROFILE = None  # BassKernelResults of the last run (for test harness)

_PROG_CACHE = {}


def _fit_cheb(f, lo, hi, tol, max_deg=15):
    """Fit f on [lo, hi]; return power-basis coeffs in w = 2(d-lo)/(hi-lo)-1.

    Returned length is even (odd degree, zero-padded if needed).
    """
    from numpy.polynomial import chebyshev as C

    dd = np.linspace(lo, hi, 4001)
    ff = f(dd)
    ch = None
    for deg in [2] + list(range(3, max_deg + 1, 2)):
        ch = C.Chebyshev.fit(dd, ff, deg, domain=[lo, hi])
        if np.abs(ch(dd) - ff).max() < tol:
            break
    cw = C.cheb2poly(ch.coef)
    if len(cw) % 2:
        cw = np.append(cw, 0.0)
    return cw


class _PolyEmitter:
    """Estrin evaluation of sum_k cw[k] w^k over column slices of a grid.

    Input tile holds v = w + 1 (if in_is_v) or w directly. Powers of w^2
    go on ACT (Square) when use_act is set, else DVE tensor_mul.
    """

    def __init__(self, nc, mybir, pool, shape, cw, pfx, in_is_v, use_act,
                 neg1=None, zero=None):
        self.nc, self.mybir, self.pool = nc, mybir, pool
        self.shape, self.cw, self.pfx = shape, cw, pfx
        self.in_is_v, self.use_act = in_is_v, use_act
        self.neg1, self.zero = neg1, zero
        self.K = len(cw) // 2
        f32 = mybir.dt.float32
        self.tiles = {}
        nlv = 1
        k = self.K
        while k > 1:
            k = (k + 1) // 2
            nlv += 1

        def t(name):
            self.tiles[name] = pool.tile(
                shape, f32, tag=f"{pfx}{name}", name=f"{pfx}{name}"
            )

        for i in range(self.K):
            t(f"L{i}")
        lv, cnt = 1, self.K
        while cnt > 1:
            t(f"p{lv}")
            for i in range(0, cnt - 1, 2):
                t(f"q{lv}_{i}")
            cnt = (cnt + 1) // 2
            lv += 1

    def emit(self, v_tile, sl, final_out=None, eng=None, act_t0=False):
        """Emit ops for column slice sl; returns the tile holding the result.

        final_out: optional tile the last combine writes to (e.g. a bf16
        tile). eng: engine for the fast-cubic path (nc.vector or nc.gpsimd).
        Records the last ACT instruction in self.last_act_inst (None when
        the fast path is used — it needs no ACT at all).
        """
        nc, mybir, cw = self.nc, self.mybir, self.cw
        Alu = mybir.AluOpType
        Act = mybir.ActivationFunctionType
        T = self.tiles
        self.last_act_inst = None
        if eng is None:
            eng = nc.vector
        if (self.K == 2 and self.in_is_v and float(cw[0]) == 0.0
                and float(cw[3]) == 0.0):
            # centered quadratic: d_eta = (v-1) * (c1 + c2*(v-1)); no ACT
            c1, c2 = float(cw[1]), float(cw[2])
            A = T["L0"]
            nc.vector.tensor_scalar(A[:, sl], v_tile[:, sl], c2, c1 - c2,
                                    Alu.mult, Alu.add)
            dst = final_out if final_out is not None else T["q1_0"]
            nc.vector.scalar_tensor_tensor(dst[:, sl], v_tile[:, sl], 1.0,
                                           A[:, sl], Alu.subtract, Alu.mult)
            return dst
        if (self.K == 2 and self.in_is_v and self.use_act
                and float(cw[0]) == 0.0):
            # centered cubic: d_eta = (v-1) * (c1 + c2*(v-1) + c3*(v-1)^2)
            # w2 = (v-1)^2 on ACT; 3 ops on DVE
            c1, c2, c3 = float(cw[1]), float(cw[2]), float(cw[3])
            p = T["p1"]
            self.last_act_inst = nc.scalar.activation(
                p[:, sl], v_tile[:, sl], Act.Square,
                bias=self.neg1[: self.shape[0]],
            )
            t0 = T["L0"]
            self.t0_act_inst = None
            if act_t0:
                # ACT is idle after this half's Square; DVE is still busy
                self.t0_act_inst = nc.scalar.activation(
                    t0[:, sl], p[:, sl], Act.Copy,
                    bias=float(c1 - c2), scale=c3,
                )
            else:
                nc.vector.tensor_scalar(t0[:, sl], p[:, sl], c3, c1 - c2,
                                        Alu.mult, Alu.add)
            p2t = T["L1"]
            nc.vector.scalar_tensor_tensor(p2t[:, sl], v_tile[:, sl], c2,
                                           t0[:, sl], Alu.mult, Alu.add)
            dst = final_out if final_out is not None else T["q1_0"]
            nc.vector.scalar_tensor_tensor(dst[:, sl], v_tile[:, sl], 1.0,
                                           p2t[:, sl], Alu.subtract, Alu.mult)
            return dst
        cur = []
        for k in range(self.K):
            L = T[f"L{k}"]
            c1 = float(cw[2 * k + 1])
            c0 = float(cw[2 * k] - cw[2 * k + 1]) if self.in_is_v else float(cw[2 * k])
            nc.vector.tensor_scalar(L[:, sl], v_tile[:, sl], c1, c0, Alu.mult, Alu.add)
            cur.append(L)
        if self.K == 1:
            return cur[0]
        p = T["p1"]
        if self.use_act:
            bias = self.neg1 if self.in_is_v else self.zero
            self.last_act_inst = nc.scalar.activation(
                p[:, sl], v_tile[:, sl], Act.Square, bias=bias[: self.shape[0]]
            )
        else:
            if self.in_is_v:
                w = self.pool.tile(self.shape, self.mybir.dt.float32, tag=f"{self.pfx}w")
                nc.vector.tensor_scalar(w[:, sl], v_tile[:, sl], 1.0, -1.0, Alu.mult, Alu.add)
                nc.vector.tensor_mul(p[:, sl], w[:, sl], w[:, sl])
            else:
                nc.vector.tensor_mul(p[:, sl], v_tile[:, sl], v_tile[:, sl])
        lv = 1
        while len(cur) > 1:
            nxt = []
            last_level = len(cur) <= 2
            for i in range(0, len(cur) - 1, 2):
                q = T[f"q{lv}_{i}"]
                nc.vector.tensor_mul(q[:, sl], p[:, sl], cur[i + 1][:, sl])
                dst = final_out if (last_level and final_out is not None) else q
                nc.vector.tensor_add(dst[:, sl], cur[i][:, sl], q[:, sl])
                nxt.append(dst)
            if len(cur) % 2:
                nxt.append(cur[-1])
            cur = nxt
            if len(cur) > 1:
                p2 = T[f"p{lv + 1}"]
                if self.use_act:
                    self.last_act_inst = nc.scalar.activation(
                        p2[:, sl], p[:, sl], Act.Square, bias=self.zero[: self.shape[0]]
                    )
                else:
                    nc.vector.tensor_mul(p2[:, sl], p[:, sl], p[:, sl])
                p = p2
                lv += 1
        return cur[0]


def _build(cw_eta, cw_xi, s2):
    import concourse.bacc as bacc
    import concourse.bass as bass
    import concourse.mybir as mybir
    from concourse import tile

    f32 = mybir.dt.float32
    Alu = mybir.AluOpType
    Act = mybir.ActivationFunctionType

    nc = bacc.Bacc("TRN2", target_bir_lowering=False, debug=False)
    bf16 = mybir.dt.bfloat16
    # MM data: 4 quadrants x 16 rows (13 used: hi/lo split Gram operands)
    mm_d = nc.declare_dram_parameter("mm", [64, 384], bf16, isOutput=False)
    # extras: xt4 (rows 0-3, cols 0:128), wxi (row 0, 128:256), K (rows 0-3, col 256)
    ext_d = nc.declare_dram_parameter("ext", [8, 257], f32, isOutput=False)
    xaug_d = nc.declare_dram_parameter("xaug_r", [PB, 4 * NCORES], bf16, isOutput=False)
    out_d = nc.declare_dram_parameter("res", [4, PB], f32, isOutput=True)

    with tile.TileContext(nc) as tc:
        with (
            tc.tile_pool(name="sb", bufs=1) as sb,
            tc.tile_pool(name="ps", bufs=1, space=bass.MemorySpace.PSUM) as ps,
        ):
            mmt = sb.tile([128, 384], bf16, tag="mmt")
            dma_engines = [nc.sync, nc.scalar, nc.gpsimd, nc.sync]
            for g in range(4):
                dma_engines[g].dma_start(
                    mmt[32 * g:32 * g + 16, :],
                    mm_d[16 * g:16 * g + 16, :],
                )
            # extras (xt/wxi/K) needed late
            ext = sb.tile([8, 257], f32, tag="ext")
            nc.sync.dma_start(ext[:], ext_d[:])
            xaug = sb.tile([PB, 4 * NCORES], bf16, tag="xaug")
            nc.scalar.dma_start(xaug[:], xaug_d[:])
            xt_sl = slice(0, 128)
            wxi_sl = slice(128, 256)
            k_sl = slice(256, 257)

            neg1 = sb.tile([128, 1], f32, tag="neg1")
            nc.vector.memset(neg1[:], -1.0)
            zero = sb.tile([128, 1], f32, tag="zero")
            nc.vector.memset(zero[:], 0.0)
            ones4 = sb.tile([1, 4], f32, tag="ones4")
            nc.vector.memset(ones4[:], 1.0)

            # dist^2 grid via 4x row tiling: quadrant g computes chunks g
            # and g+4 into its own PSUM bank; v cols = g*256 + (b//4)*128 + il
            d2t0 = ps.tile([128, 256], f32, tag="d2t0")
            d2t1 = ps.tile([128, 256], f32, tag="d2t1")
            d2t2 = ps.tile([128, 256], f32, tag="d2t2")
            d2t3 = ps.tile([128, 256], f32, tag="d2t3")
            d2 = [d2t0, d2t1, d2t2, d2t3]
            for g in range(4):
                for half in range(2):
                    # quadrant-grouped order [0,4],[1,5],.. so bank g is
                    # complete as early as possible for the sqrt chain.
                    # bf16 hi/lo-split operands: K=13, single HW pass.
                    nc.tensor.matmul(
                        d2[g][:, half * PB:(half + 1) * PB],
                        mmt[32 * g:32 * g + 13, half * 128:half * 128 + 128],
                        mmt[32 * g:32 * g + 13, 256:384],
                        start=True, stop=True,
                        tile_position=(32 * g, 0),
                    )

            v = sb.tile([128, N], f32, tag="v")
            etab = sb.tile([128, N], bf16, tag="etab")  # delta-eta in bf16
            # ts4 rows: 0 = S_delta (ones col), 1-3 = T_delta (x cols)
            ts4 = ps.tile([4, PB], f32, tag="ts4")
            pe = _PolyEmitter(nc, mybir, sb, [128, N], cw_eta, "e",
                              in_is_v=True, use_act=True, neg1=neg1, zero=zero)
            prev_act = [None]

            def act_chain(inst):
                # pin ACT queue order (FIFO engine; Tile otherwise reorders)
                if prev_act[0] is not None:
                    tile.add_dep_helper(inst.ins, prev_act[0].ins, sync=False)
                prev_act[0] = inst

            first_ts = True
            for h in range(2):
                for g in (2 * h, 2 * h + 1):
                    # sqrt per bank (one [128,256] op per PSUM bank)
                    si = nc.scalar.activation(
                        v[:, g * 256:(g + 1) * 256], d2[g][:], Act.Sqrt,
                        bias=zero[:], scale=float(s2))
                    act_chain(si)
                # poly per half: wider DVE ops amortize per-op overhead
                pe.emit(v, slice(h * 512, (h + 1) * 512), final_out=etab,
                        act_t0=(h == 1))
                if pe.last_act_inst is not None:
                    act_chain(pe.last_act_inst)
                if getattr(pe, "t0_act_inst", None) is not None:
                    act_chain(pe.t0_act_inst)
                for g in (2 * h, 2 * h + 1):
                    for half in range(2):
                        b = g + 4 * half  # j-chunk index
                        col = g * 256 + half * 128
                        last = (h == 1 and g == 3 and half == 1)
                        nc.tensor.matmul(
                            ts4[:],
                            xaug[:, 4 * b:4 * (b + 1)],
                            etab[:, col:col + PB],
                            start=first_ts, stop=last,
                        )
                        first_ts = False

            # xi poly on [1, 128] (input is w directly)
            g = sb.tile([1, PB], f32, tag="g")
            if (len(cw_xi) == 4 and float(cw_xi[3]) == 0.0
                    and float(cw_xi[2]) > 0.0):
                # complete the square: xi = c2*(w+h)^2 + k — ONE op on the
                # otherwise-idle ACT engine; k folds into the g combine
                c0, c1, c2 = (float(cw_xi[0]), float(cw_xi[1]),
                              float(cw_xi[2]))
                sq = c2 ** 0.5
                hh = c1 / (2.0 * c2)
                kk = c0 - c1 * c1 / (4.0 * c2)
                sxh = sb.tile([1, 1], f32, tag="sxh")
                nc.vector.memset(sxh[:], sq * hh)
                xsq = sb.tile([1, PB], f32, tag="xsq")
                si = nc.scalar.activation(xsq[:], ext[0:1, wxi_sl],
                                          Act.Square, bias=sxh[:], scale=sq)
                act_chain(si)
                # g = (xi_sq + k) + S   (S is ts4 row 0 — partition 0)
                nc.vector.scalar_tensor_tensor(g[:], xsq[:], kk,
                                               ts4[0:1, :], Alu.add, Alu.add)
            elif len(cw_xi) == 4 and float(cw_xi[3]) == 0.0:
                # quadratic xi on DVE (c2 <= 0 fallback)
                wv = ext[0:1, wxi_sl]
                xA = sb.tile([1, PB], f32, tag="xA")
                nc.vector.tensor_scalar(xA[:], wv, float(cw_xi[2]),
                                        float(cw_xi[1]), Alu.mult, Alu.add)
                xB = sb.tile([1, PB], f32, tag="xB")
                nc.vector.scalar_tensor_tensor(xB[:], wv, 1.0, xA[:],
                                               Alu.mult, Alu.mult)
                # g = (xB + c0) + S   (S is ts4 row 0 — partition 0)
                nc.vector.scalar_tensor_tensor(g[:], xB[:], float(cw_xi[0]),
                                               ts4[0:1, :], Alu.add, Alu.add)
            else:
                wxi = sb.tile([1, PB], f32, tag="wxi")
                nc.vector.tensor_copy(wxi[:], ext[0:1, wxi_sl])
                px = _PolyEmitter(nc, mybir, sb, [1, PB], cw_xi, "x",
                                  in_is_v=False, use_act=False)
                xi_t = px.emit(wxi, slice(0, PB))
                # g = xi + S  (S is ts4 row 0 — partition 0, legal to read)
                nc.vector.tensor_add(g[:], xi_t[:, 0:PB], ts4[0:1, :])
            # broadcast g over 4 partitions in PSUM
            sb4 = ps.tile([4, PB], f32, tag="sb4")
            nc.tensor.matmul(sb4[:], ones4[:], g[:], start=True, stop=True)
            # res rows 1-3 = g * x - T; row 0 = -S (junk, host drops it)
            res = sb.tile([4, PB], f32, tag="res")
            nc.vector.tensor_mul(res[:], sb4[:], ext[0:4, xt_sl])
            # res = (g*x - K) - T_delta ; K = eta0 * sum_j x_j per row
            nc.vector.scalar_tensor_tensor(
                res[:], res[:], ext[0:4, k_sl], ts4[:],
                Alu.subtract, Alu.subtract,
            )
            nc.sync.dma_start(out_d[:], res[:])

    nc.finalize()
    return nc


def _host_prep(x, r2_32, eps):
    aug_j = np.zeros((8, N), np.float32)
    aug_j[0:3] = -2.0 * x.T
    aug_j[3] = r2_32 + np.float32(eps)
    aug_j[4] = 1.0
    xaug_r = np.zeros((PB, 4 * NCORES), np.float32)
    for b in range(NCORES):
        xaug_r[:, 4 * b] = 1.0
        xaug_r[:, 4 * b + 1:4 * b + 4] = x[b * PB:(b + 1) * PB]
    return aug_j, xaug_r


def kernel(**inputs):
    global LAST_PROFILE
    x = np.ascontiguousarray(np.asarray(inputs["x"], dtype=np.float32))
    t = float(np.asarray(inputs["t"]))
    W = {
        k: np.asarray(v, np.float64)
        for k, v in inputs.items()
        if k not in ("x", "t")
    }

    def mlp(inp, p):
        sp = lambda z: np.logaddexp(0.0, z)
        h = sp(inp @ W[p + "_W1"] + W[p + "_b1"])
        h = sp(h @ W[p + "_W2"] + W[p + "_b2"])
        return h @ W[p + "_W3"] + W[p + "_b3"]

    def eta_f(dd):
        return mlp(np.stack([dd, np.full_like(dd, t)], -1), "eta")[..., 0]

    def xi_f(rr):
        return mlp(np.stack([rr, np.full_like(rr, t)], -1), "xi")[..., 0]

    r2_32 = (x * x).sum(1, dtype=np.float32)
    r64 = np.sqrt(r2_32.astype(np.float64))
    # eps shift keeps the PE-rounded diagonal of dist^2 positive (no relu).
    r2max = float(r2_32.max())
    # covers PE rounding + bf16 hi/lo split residuals (xl*xl term dropped)
    eps = max(2e-4 * max(r2max, 1.0), 1e-30)
    dmax = np.sqrt((2.0 * float(r64.max())) ** 2 + 2 * eps) * 1.0001 + 1e-12
    rlo = float(r64.min()) * 0.999 - 1e-12
    rhi = float(r64.max()) * 1.001 + 1e-12

    eta_scale = np.abs(eta_f(np.linspace(0, dmax, 257))).max()
    tol_eta = max(eta_scale * 1e-7, 1e-10)
    tol_xi = max(np.abs(xi_f(np.linspace(rlo, rhi, 257))).max() * 1e-7, 1e-10)
    # Guard for the eps shift: worst-case |eta'| * max d-shift must be tiny.
    dgrid = np.linspace(1e-3, dmax, 2049)
    deta = np.abs(np.gradient(eta_f(dgrid), dgrid)).max()
    dmin_guard = 1e-2  # conservative lower bound on off-diag distances
    shift_err = deta * eps / (2.0 * dmin_guard)
    assert shift_err < 1e-3 * max(eta_scale, 1e-30), (
        f"eps-shift error bound {shift_err} too large; need relu fallback"
    )

    cw_eta = _fit_cheb(eta_f, 0.0, dmax, tol_eta)
    cw_xi = _fit_cheb(xi_f, rlo, rhi, tol_xi)
    s = 2.0 / dmax
    s2 = s * s

    # Center-subtract eta so the on-device grid holds delta-eta (tiny values,
    # safe in bf16). The constant part is restored exactly: N*eta0 into the
    # xi constant (feeds S), eta0*sum_j x_j via the K column (feeds T).
    eta0 = float(cw_eta[0])
    cw_eta_dev = cw_eta.copy()
    cw_eta_dev[0] = 0.0
    cw_xi_dev = cw_xi.copy()
    cw_xi_dev[0] += N * eta0

    key = (cw_eta_dev.tobytes(), cw_xi_dev.tobytes(), float(s2))
    nc = _PROG_CACHE.get(key)
    if nc is None:
        nc = _build(cw_eta_dev, cw_xi_dev, s2)
        _PROG_CACHE[key] = nc

    aug_j, xaug_r = _host_prep(x, r2_32, eps)
    import ml_dtypes
    bf = ml_dtypes.bfloat16
    xaug_r = xaug_r.astype(bf)
    w_xi_full = (2.0 * (r64 - rlo) / (rhi - rlo) - 1.0).astype(np.float32)
    ksum = (eta0 * x.astype(np.float64).sum(0)).astype(np.float32)  # [3]
    # bf16 hi/lo splits for the single-pass Gram matmul
    xh = x.astype(bf)
    xl = (x - xh.astype(np.float32)).astype(bf)
    xh2 = (-2.0 * xh.astype(np.float32)).astype(bf)  # exact (exponent shift)
    xl2 = (-2.0 * xl.astype(np.float32)).astype(bf)
    r2e = (r2_32 + np.float32(eps)).astype(np.float32)
    r2eh = r2e.astype(bf)
    r2el = (r2e - r2eh.astype(np.float32)).astype(bf)
    r2h = r2_32.astype(bf)
    r2l = (r2_32 - r2h.astype(np.float32)).astype(bf)

    in_maps = []
    for m in range(NCORES):
        sl = slice(m * PB, (m + 1) * PB)
        mm = np.zeros((64, 384), bf)
        for g in range(4):
            R = 16 * g
            for half, c in ((0, g), (1, g + 4)):
                cs = slice(c * PB, (c + 1) * PB)
                col = slice(half * 128, (half + 1) * 128)
                mm[R + 0:R + 3, col] = xh2[cs].T
                mm[R + 3:R + 6, col] = xh2[cs].T
                mm[R + 6:R + 9, col] = xl2[cs].T
                mm[R + 9, col] = r2eh[cs]
                mm[R + 10, col] = r2el[cs]
                mm[R + 11, col] = 1.0
                mm[R + 12, col] = 1.0
            mm[R + 0:R + 3, 256:384] = xh[sl].T
            mm[R + 3:R + 6, 256:384] = xl[sl].T
            mm[R + 6:R + 9, 256:384] = xh[sl].T
            mm[R + 9, 256:384] = 1.0
            mm[R + 10, 256:384] = 1.0
            mm[R + 11, 256:384] = r2h[sl]
            mm[R + 12, 256:384] = r2l[sl]
        ext = np.zeros((8, 257), np.float32)
        ext[1:4, 0:128] = x[sl].T
        ext[0, 128:256] = w_xi_full[sl]
        ext[1:4, 256] = ksum
        in_maps.append({"mm": mm, "ext": ext, "xaug_r": xaug_r})

    from concourse.bass_utils import run_bass_kernel_spmd

    kw = {}
    if TRACE:
        kw = dict(trace=True, tmpdir=TRACE_DIR)
    out = run_bass_kernel_spmd(nc, in_maps, list(range(NCORES)), **kw)
    LAST_PROFILE = out
    res = np.concatenate(
        [out.results[m]["res"][1:4, :].T for m in range(NCORES)], axis=0
    )
    return np.ascontiguousarray(res).astype(np.float32)
